# revision 1
# baseline (speedup 1.0000x reference)
"""TRN2 Bass kernel: differentiable palette quantization (soft VQ).

  weights = softmax_k( -|x - p_k|^2 / T );  out = sum_k weights_k p_k

Data-parallel over 8 NeuronCores (4 images each). Uses the softmax
shift-invariance  -|x-p|^2 ~ 2 x.p - |p|^2  so both heavy stages are
PE matmuls; exp on ACT; normalization via DMA row-regrouping + DVE.

Self-contained: includes the walrus sync-wait-limit workaround (this
toolchain allows ONE semaphore wait per instruction) and the Tile exit
drain patch.
"""

import sys

sys.path.insert(0, "/opt/trn_rl_repo")

import numpy as np

import concourse.bass as bass
import concourse.tile as tile
from concourse import mybir
from concourse.tile import ScopedClock

B, H, W, C, K = 32, 256, 256, 3, 32
NPIX = H * W
N_CORES = 8
IMGS = B // N_CORES     # 4 images per core
BLK = 4096              # pixels per block
NBLK = NPIX // BLK      # 16 blocks per image
NBLK_CORE = IMGS * NBLK  # 64
NGRP = NBLK_CORE // 4    # 16 groups (4 blocks each) per core
F32 = mybir.dt.float32
F16 = mybir.dt.float16

# ---------------------------------------------------------------------------
# Toolchain workarounds
# ---------------------------------------------------------------------------

_MAX_WAITS = 1


def _split_excess_waits(nc):
    """This walrus build rejects >1 sync wait per instruction. Move the
    excess onto same-engine NOPs inserted immediately before."""
    for f in nc.m.functions:
        for bb in f.blocks:
            insts = bb.instructions
            if not any(
                i.sync_info is not None and len(i.sync_info.on_wait) > _MAX_WAITS
                for i in insts
            ):
                continue
            new = []
            for inst in insts:
                si = inst.sync_info
                waits = list(si.on_wait) if si is not None else []
                if len(waits) > _MAX_WAITS:
                    extra, keep = waits[:-_MAX_WAITS], waits[-_MAX_WAITS:]
                    for i in range(0, len(extra), _MAX_WAITS):
                        new.append(
                            mybir.InstNoOp(
                                name=nc.get_next_instruction_name(),
                                engine=inst.engine,
                                bass_nofuse=True,
                                sync_info=mybir.SyncInfo(
                                    on_wait=extra[i : i + _MAX_WAITS], on_update=[]
                                ),
                            )
                        )
                    inst.sync_info = mybir.SyncInfo(
                        on_wait=keep, on_update=list(si.on_update)
                    )
                new.append(inst)
            bb.instructions = new


def _patched_drain_and_barrier(self, tick_clock, wait_clock):
    """Tile's exit drain carries one wait per active proc; spread them
    across single-wait NOPs (same walrus limit as above)."""
    nc = self.nc
    probe = nc.sync.nop(nofuse=True, hint="drain_waits")
    wait_clock.add_sem_waits(probe.ins, ScopedClock({None: tick_clock.global_clock}))
    si = probe.ins.sync_info
    waits = list(si.on_wait) if si is not None else []
    updates = list(si.on_update) if si is not None else []
    if len(waits) > 1:
        probe.ins.sync_info = mybir.SyncInfo(on_wait=waits[:1], on_update=updates)
        for i, w in enumerate(waits[1:]):
            extra = nc.sync.nop(nofuse=True, hint=f"drain_waits_{i}")
            extra.ins.sync_info = mybir.SyncInfo(on_wait=[w], on_update=[])
    nc.sync.drain()
    nc.all_engine_barrier()
    assert self.sems is not None
    popped = nc._tile_sem_poison_stack.pop()
    assert popped is self._sem_poison
    nc.clear_and_free_semaphores(list(self.sems.allocated().values()))
    nc.all_engine_barrier()


tile.TileContext._drain_and_barrier = _patched_drain_and_barrier

# ---------------------------------------------------------------------------
# Program builder
# ---------------------------------------------------------------------------


def _build_program():
    nc = bass.Bass()
    x = nc.dram_tensor("x", [IMGS, NBLK // 4, 12, 4096], F16, kind="ExternalInput")
    w1 = nc.dram_tensor("w1", [12, 128 * IMGS], F16, kind="ExternalInput")
    w2 = nc.dram_tensor("w2", [128, 128 * IMGS], F16, kind="ExternalInput")
    bias = nc.dram_tensor("bias", [128, IMGS], F32, kind="ExternalInput")
    out = nc.dram_tensor("out", [NGRP, 128, 1024], F32, kind="ExternalOutput")

    with tile.TileContext(nc) as tc:
        with (
            tc.tile_pool(name="singles", bufs=1) as singles,
            tc.tile_pool(name="xt", bufs=3) as xt_pool,
            tc.tile_pool(name="e", bufs=3) as e_pool,
            tc.tile_pool(name="p1", bufs=2, space="PSUM") as p1_pool,
            tc.tile_pool(name="p2", bufs=2, space="PSUM") as p2_pool,
            tc.tile_pool(name="y", bufs=8) as y_pool,
            tc.tile_pool(name="sd", bufs=2) as sd_pool,
            tc.tile_pool(name="r", bufs=2) as r_pool,
            tc.tile_pool(name="rb", bufs=3) as rb_pool,
            tc.tile_pool(name="o", bufs=3) as o_pool,
        ):
            w1sb = singles.tile([12, 128 * IMGS], F16)
            w2sb = singles.tile([128, 128 * IMGS], F16)
            bsb = singles.tile([128, IMGS], F32)
            nc.sync.dma_start(out=w1sb[:], in_=w1[:])
            nc.scalar.dma_start(out=w2sb[:], in_=w2[:])
            nc.gpsimd.dma_start(out=bsb[:], in_=bias[:])

            def load_group(gg):
                img, gi = divmod(gg, NBLK // 4)
                xt = xt_pool.tile([12, 4096], F16, name="xt")
                nc.sync.dma_start(out=xt[:], in_=x[img, gi])
                return xt

            def mm1(bb, xt):
                img = bb // NBLK
                q = (bb % NBLK) % 4
                p1 = p1_pool.tile([128, 1024], F32, name="p1")
                for j in range(2):
                    nc.tensor.matmul(
                        out=p1[:, 512 * j : 512 * j + 512],
                        lhsT=w1sb[:, 128 * img : 128 * img + 128],
                        rhs=xt[:, 1024 * q + 512 * j : 1024 * q + 512 * j + 512],
                        start=True,
                        stop=True,
                        tile_position=(0, 0),
                    )
                return p1

            p2 = None
            sd = None
            pend = []
            deferred = []
            xt_cur = load_group(0)
            xt_pre = load_group(1)
            xt_fut = None
            p1_cur = mm1(0, xt_cur)
            for bb in range(NBLK_CORE):
                img, b = divmod(bb, NBLK)
                q = b % 4
                if bb % 4 == 1 and bb // 4 + 2 <= NGRP - 1:
                    xt_fut = load_group(bb // 4 + 2)
                if bb + 1 < NBLK_CORE:
                    if (bb + 1) % 4 == 0:
                        xt_nxt = xt_pre
                        xt_pre = xt_fut
                    else:
                        xt_nxt = xt_cur
                    p1_nxt = mm1(bb + 1, xt_nxt)

                e = e_pool.tile([128, 1024], F16, name="e")
                nc.scalar.activation(
                    out=e[:],
                    in_=p1_cur[:],
                    func=mybir.ActivationFunctionType.Exp,
                    bias=bsb[:, img : img + 1],
                    scale=1.0,
                )

                if q == 0:
                    p2 = p2_pool.tile([128, 1024], F32, name="p2")
                for ip in range(2):
                    g = (q + 2 * ip) % 4
                    nc.tensor.matmul(
                        out=p2[32 * g : 32 * g + 16, 512 * ip : 512 * ip + 512],
                        lhsT=w2sb[:, 128 * img + 32 * g : 128 * img + 32 * g + 16],
                        rhs=e[:, 512 * ip : 512 * ip + 512],
                        start=True,
                        stop=True,
                        tile_position=(0, 32 * g),
                    )

                if q == 3:
                    grp = bb // 4
                    m = len(pend)
                    y = y_pool.tile([128, 1024], F32, name="y")
                    nc.vector.tensor_copy(out=y[:], in_=p2[:])
                    if m == 0:
                        sd = sd_pool.tile([128, 1024], F32, name="sd")
                    src = bass.AP(
                        tensor=y.tensor,
                        offset=y.offset + 3 * 1024,
                        ap=[[4 * 1024, 32], [1, 1024]],
                    )
                    nc.gpsimd.dma_start(out=sd[32 * m : 32 * m + 32, :], in_=src)
                    pend.append((grp, m, y))
                    if grp in (3, 7, 11, 13, 14, 15):
                        r = r_pool.tile([128, 1024], F32, name="r")
                        nc.scalar.activation(
                            out=r[:],
                            in_=sd[:],
                            func=mybir.ActivationFunctionType.Ln,
                        )
                        nc.scalar.activation(
                            out=r[:],
                            in_=r[:],
                            func=mybir.ActivationFunctionType.Exp,
                            scale=-1.0,
                        )

                        def mk_unit(grp_i, mi, yi, r=r):
                            def emit():
                                rb = rb_pool.tile([128, 1024], F32, name="rb")
                                for cc in range(3):
                                    dst = bass.AP(
                                        tensor=rb.tensor,
                                        offset=rb.offset + cc * 1024,
                                        ap=[[4 * 1024, 32], [1, 1024]],
                                    )
                                    nc.gpsimd.dma_start(
                                        out=dst, in_=r[32 * mi : 32 * mi + 32, :]
                                    )
                                o = o_pool.tile([128, 1024], F32, name="o")
                                nc.vector.tensor_mul(out=o[:], in0=yi[:], in1=rb[:])
                                nc.sync.dma_start(out=out[grp_i], in_=o[:])
                            return emit

                        for grp_i, mi, yi in pend:
                            deferred.append(mk_unit(grp_i, mi, yi))
                        pend = []

                if deferred:
                    deferred.pop(0)()

                if bb + 1 < NBLK_CORE:
                    xt_cur, p1_cur = xt_nxt, p1_nxt
            while deferred:
                deferred.pop(0)()

    _split_excess_waits(nc)
    return nc


_PROGRAM = None


def _get_program():
    global _PROGRAM
    if _PROGRAM is None:
        _PROGRAM = _build_program()
    return _PROGRAM


# ---------------------------------------------------------------------------
# Host-side prep / decode
# ---------------------------------------------------------------------------


def _prep_core_inputs(images4, palettes4, temperature):
    T = float(temperature)
    im = images4.reshape(IMGS, NBLK // 4, 4, 4, 1024, C).transpose(0, 1, 3, 5, 2, 4)
    # axes [img, grp, u, c, q, n]; row = 3u+c, free = 1024q + n
    xs = np.ascontiguousarray(im.reshape(IMGS, NBLK // 4, 12, 4096)).astype(np.float16)
    w1 = np.zeros((12, 128 * IMGS), np.float16)
    w2 = np.zeros((128, 128 * IMGS), np.float16)
    bias = np.zeros((128, IMGS), np.float32)
    for img in range(IMGS):
        pal = palettes4[img].astype(np.float32)
        pt = ((2.0 / T) * pal.T).astype(np.float16)
        pal16 = pal.astype(np.float16)
        bvec = -(pal * pal).sum(axis=1) / T
        for u in range(4):
            w1[3 * u : 3 * u + 3, 128 * img + 32 * u : 128 * img + 32 * u + 32] = pt
            bias[32 * u : 32 * u + 32, img] = bvec
            for g in range(4):
                col = 128 * img + 32 * g + 4 * u
                w2[32 * u : 32 * u + 32, col : col + 3] = pal16
                w2[32 * u : 32 * u + 32, col + 3] = 1.0
    return {"x": xs, "w1": w1, "w2": w2, "bias": bias}


def _decode_core_output(out_core):
    """out[grp, 16g+4u+c, 512ip+n] -> [IMGS, NPIX, C].
    img = grp//4, q = (g-2ip)%4, pixel = 4096(4(grp%4)+q) + 1024u + 512ip + n.
    """
    o = out_core.reshape(NGRP, 4, 2, 4, 4, 2, 512)[:, :, 0, :, :, :, :]
    # [grp, g, (16-row half), u, c, ip, n] -> valid half only
    res = np.empty((IMGS, NPIX, C), np.float32)
    for g in range(4):
        for ip in range(2):
            q = (g - 2 * ip) % 4
            for grp in range(NGRP):
                img, gi = divmod(grp, 4)
                base = 4096 * (gi * 4 + q) + 512 * ip
                src = o[grp, g, :, :3, ip, :]  # [u, c, n]
                for u in range(4):
                    p0 = base + 1024 * u
                    res[img, p0 : p0 + 512, :] = src[u].T
    return res


# ---------------------------------------------------------------------------
# Entry points
# ---------------------------------------------------------------------------


def run(images, palettes, temperature, trace=False):
    """Returns (output [B,H,W,C] f32, exec_time_ns or None)."""
    from concourse.bass_utils import run_bass_kernel_spmd

    images = np.asarray(images, np.float32)
    palettes = np.asarray(palettes, np.float32)
    nc = _get_program()
    in_maps = [
        _prep_core_inputs(
            images[IMGS * c : IMGS * (c + 1)],
            palettes[IMGS * c : IMGS * (c + 1)],
            temperature,
        )
        for c in range(N_CORES)
    ]
    res = run_bass_kernel_spmd(nc, in_maps, list(range(N_CORES)), trace=trace)
    outs = [_decode_core_output(res.results[c]["out"]) for c in range(N_CORES)]
    full = np.concatenate(outs, axis=0).reshape(B, H, W, C)
    return full, res.exec_time_ns


def kernel(images, palettes, temperature):
    return run(images, palettes, temperature)[0]

